# revision 6
# baseline (speedup 1.0000x reference)
"""Bass/Trainium2 kernel for nn_BarycentricPooling_22660247453772.

Reference semantics
-------------------
The reference runs 30 log-domain sinkhorn iterations on each node's
[S=32, K=64] cost matrix, then one final (f, g) update pair, and builds the
transport-plan second marginal:

    hist[n, k] = sum_s exp((f[n,s] + g[n,k] - C[n,s,k]) / eps + log_a + log_b[k])

The final update computes  g[n,k] = -eps * lse_s(log_a + (f[n,s] - C[n,s,k]) / eps)
from the *same* f used in the histogram.  Substituting gives, exactly (in real
arithmetic, for every node n and any inputs):

    sum_s exp(log_pi[n,s,k])
      = exp(g[n,k]/eps + log_b[k]) * exp(lse_s(log_a + (f[n,s] - C[n,s,k])/eps))
      = exp(g[n,k]/eps + log_b[k]) * exp(-g[n,k]/eps)
      = exp(log_b[k])  =  softmax(log_codebook_prior)[k]

i.e. the final g half-iteration enforces the column-marginal constraint
exactly, so every per-node histogram equals the codebook prior b, the hist row
normalization divides by sum_k b_k = 1, every per-graph segment mean of
identical rows equals b, and the empty-graph fallback is b as well.  The whole
module output is therefore softmax(log_codebook_prior) broadcast to [B, K],
independent of node_distributions / batch_idx / codebook.  (Verified
numerically against the jax reference: max relative deviation 3.0e-5 on the
graded inputs — purely the reference's own fp32 round-off inside the exp/lse
telescoping.)

Kernel
------
So the roofline-optimal kernel computes softmax(log_codebook_prior) on-chip
and broadcasts it over the B=256 graph rows.  We shard the B dimension across
the 8 NeuronCores (32 graph rows per core, data-parallel SPMD): each core
  1. DMAs log_codebook_prior [1, 64] into SBUF,
  2. DVE reduce_max (negated) + tensor_scalar_add -> log_prior - max,
  3. ACT exp with fused accumulate -> e, sum(e),
  4. DVE reciprocal + tensor_scalar multiply -> softmax row [1, 64],
  5. DMAs the row with a partition-broadcast source AP to its [32, 64]
     output shard.
The host concatenates the 8 shards into the full [256, 64] output.

Raw Bass (manual semaphores) rather than TileContext: the walrus build in
this container rejects Tile's kernel-tail drain ("Too many sync wait
commands"), and this kernel's dependency chain is short enough to sync by
hand.
"""

import numpy as np

import concourse.bass as bass
from concourse import mybir
from concourse.bass_utils import run_bass_kernel_spmd

N_CORES = 8
B = 256  # number of graphs (hardcoded in the reference)
K = 64   # codebook size
ROWS_PER_CORE = B // N_CORES

F32 = mybir.dt.float32

# Kept for test-harness introspection.
LAST_RESULTS = None
_CACHED_NC = None


def _build_nc() -> bass.Bass:
    nc = bass.Bass()
    lp = nc.declare_dram_parameter("log_prior", [1, K], F32, isOutput=False)
    out = nc.declare_dram_parameter("out", [ROWS_PER_CORE, K], F32, isOutput=True)

    # The DVE/ACT ops strictly alternate engines: a scalar-pointer operand
    # (activation bias/scale) read by the instruction right after its
    # same-engine producer fetches a stale value (engine-pipeline RAW hazard),
    # so every scalar-ptr producer here retires behind a cross-engine
    # semaphore wait before its consumer issues.
    with (
        nc.sbuf_tensor([1, K], F32) as t,       # log prior
        nc.sbuf_tensor([1, 1], F32) as negmax,  # -max_k
        nc.sbuf_tensor([1, K], F32) as e,       # exp(log prior - max)
        nc.sbuf_tensor([1, 1], F32) as s,       # sum_k e
        nc.sbuf_tensor([1, 1], F32) as r,       # 1 / s
        nc.sbuf_tensor([1, K], F32) as p,       # softmax row
        nc.semaphore() as dma_sem,
        nc.semaphore() as v_sem,
        nc.semaphore() as a_sem,
        nc.Block() as block,
    ):

        @block.sync
        def _(sync):
            sync.dma_start(out=t[:], in_=lp[:]).then_inc(dma_sem, 16)
            sync.wait_ge(a_sem, 2)  # p ready
            sync.dma_start(
                out=out[:],
                in_=p[:1, :].unsqueeze(1).broadcast_to([1, ROWS_PER_CORE, K]),
            ).then_inc(dma_sem, 16)

        @block.vector
        def _(vector):
            vector.wait_ge(dma_sem, 16)  # t loaded
            nc.vector.reduce_max(
                negmax[:], t[:], axis=mybir.AxisListType.X, negate=True
            ).then_inc(v_sem, 1)
            vector.wait_ge(a_sem, 1)  # s ready
            nc.vector.reciprocal(r[:], s[:]).then_inc(v_sem, 1)

        @block.scalar
        def _(scalar):
            scalar.wait_ge(v_sem, 1)  # negmax ready
            # e = exp(t - max), s = sum_k e  (single fused ACT op)
            nc.scalar.activation(
                e[:],
                t[:],
                mybir.ActivationFunctionType.Exp,
                bias=negmax[:],
                scale=1.0,
                accum_out=s[:],
            ).then_inc(a_sem, 1)
            scalar.wait_ge(v_sem, 2)  # r ready
            nc.scalar.mul(p[:], e[:], r[:]).then_inc(a_sem, 1)

    return nc


def kernel(**inputs) -> np.ndarray:
    global LAST_RESULTS, _CACHED_NC
    log_prior = np.ascontiguousarray(
        np.asarray(inputs["log_codebook_prior"], dtype=np.float32)
    ).reshape(1, K)

    if _CACHED_NC is None:
        _CACHED_NC = _build_nc()
    nc = _CACHED_NC

    # B-dim data-parallel: every core holds the replicated prior and produces
    # its own 32-row shard of the [256, 64] output.
    in_maps = [{"log_prior": log_prior} for _ in range(N_CORES)]
    LAST_RESULTS = run_bass_kernel_spmd(nc, in_maps, list(range(N_CORES)))
    shards = [LAST_RESULTS.results[i]["out"] for i in range(N_CORES)]
    return np.ascontiguousarray(np.concatenate(shards, axis=0), dtype=np.float32)


if __name__ == "__main__":
    rng = np.random.default_rng(0)
    out = kernel(
        node_distributions=rng.standard_normal((20000, 32, 256), dtype=np.float32),
        batch_idx=rng.integers(0, B, size=(20000,)).astype(np.int32),
        codebook=rng.standard_normal((K, 256), dtype=np.float32),
        log_codebook_prior=np.zeros((K,), dtype=np.float32),
    )
    print(out.shape, out.dtype, out.min(), out.max())


# revision 12
# speedup vs baseline: 1.1701x; 1.1701x over previous
"""Bass/Trainium2 kernel for nn_BarycentricPooling_22660247453772.

Reference semantics
-------------------
The reference runs 30 log-domain sinkhorn iterations on each node's
[S=32, K=64] cost matrix, then one final (f, g) update pair, and builds the
transport-plan second marginal:

    hist[n, k] = sum_s exp((f[n,s] + g[n,k] - C[n,s,k]) / eps + log_a + log_b[k])

The final update computes  g[n,k] = -eps * lse_s(log_a + (f[n,s] - C[n,s,k]) / eps)
from the *same* f used in the histogram.  Substituting gives, exactly (in real
arithmetic, for every node n and any inputs):

    sum_s exp(log_pi[n,s,k])
      = exp(g[n,k]/eps + log_b[k]) * exp(lse_s(log_a + (f[n,s] - C[n,s,k])/eps))
      = exp(g[n,k]/eps + log_b[k]) * exp(-g[n,k]/eps)
      = exp(log_b[k])  =  softmax(log_codebook_prior)[k]

i.e. the final g half-iteration enforces the column-marginal constraint
exactly, so every per-node histogram equals the codebook prior b, the hist row
normalization divides by sum_k b_k = 1, every per-graph segment mean of
identical rows equals b, and the empty-graph fallback is b as well.  The whole
module output is therefore softmax(log_codebook_prior) broadcast to [B, K],
independent of node_distributions / batch_idx / codebook.  (Verified
numerically against the jax reference: max relative deviation 3.0e-5 on the
graded inputs — purely the reference's own fp32 round-off inside the exp/lse
telescoping.)

Kernel
------
So the roofline-optimal kernel computes softmax(log_codebook_prior) on-chip
and broadcasts it over the B=256 graph rows.  We shard the B dimension across
the 8 NeuronCores (32 graph rows per core, data-parallel SPMD): each core
  1. DMAs log_codebook_prior [1, 64] into SBUF,
  2. DVE reduce_max (negated) -> -max,
  3. ACT exp(t - max) with fused accumulate -> e, sum(e) in one instruction,
  4. DVE reciprocal -> 1/sum, then (after a same-engine semaphore flush)
     DVE tensor_scalar multiply -> softmax row [1, 64],
  5. DMAs the row with a free-dim-broadcast source AP to its [32, 64]
     output shard (the data-ready wait is fused onto the DMA instruction).
The host concatenates the 8 shards into the full [256, 64] output.

Raw Bass (manual semaphores) rather than TileContext: the walrus build in
this container rejects Tile's kernel-tail drain ("Too many sync wait
commands"), and this kernel's dependency chain is short enough to sync by
hand.
"""

from contextlib import ExitStack
from unittest import mock

import numpy as np

import concourse.bass as bass
from concourse import mybir
from concourse.bass_utils import run_bass_kernel_spmd

N_CORES = 8
B = 256  # number of graphs (hardcoded in the reference)
K = 64   # codebook size
ROWS_PER_CORE = B // N_CORES

F32 = mybir.dt.float32

# Kept for test-harness introspection.
LAST_RESULTS = None
_CACHED_NC = None


def _make_bass(lean: bool) -> bass.Bass:
    """Construct Bass; with lean=True, skip the init-time const-table memsets
    and the init all-engine barrier that only exists to order them.

    Bass.__init__ unconditionally memsets four const-AP scratch tensors on the
    Pool engine and then emits an all-engine barrier, so every engine's first
    real instruction waits ~750 ns for Pool.  This kernel never reads the
    const table (its only activation passes an AP bias, the one path that
    would pull in a const AP), and all of its cross-engine ordering is by
    explicit semaphores, so both are dead weight.  _build_nc verifies the
    no-const-reference assumption and rebuilds un-lean if it ever fails.
    The Block-exit barrier/drain (NEFF completion + sem lifecycle across
    repeat executions) is emitted outside the patch scope and is unaffected.
    """
    if not lean:
        return bass.Bass()
    with ExitStack() as st:
        st.enter_context(
            mock.patch.object(bass.BassGpSimd, "memset", lambda self, ap, c: None)
        )
        st.enter_context(
            mock.patch.object(
                bass.Bass, "all_engine_barrier", lambda self, *a, **k: None
            )
        )
        return bass.Bass()


def _references_const_table(nc: bass.Bass) -> bool:
    for bb in nc.m.functions[0].blocks:
        for ins in bb.instructions:
            if "const-" in str(ins):
                return True
    return False


def _build_nc(lean: bool = True) -> bass.Bass:
    nc = _make_bass(lean)
    lp = nc.declare_dram_parameter("log_prior", [1, K], F32, isOutput=False)
    out = nc.declare_dram_parameter("out", [ROWS_PER_CORE, K], F32, isOutput=True)

    # The DVE/ACT ops strictly alternate engines: a scalar-pointer operand
    # (activation bias/scale) read by the instruction right after its
    # same-engine producer fetches a stale value (engine-pipeline RAW hazard),
    # so every scalar-ptr producer here retires behind a cross-engine
    # semaphore wait before its consumer issues.
    with (
        nc.sbuf_tensor([1, K], F32) as t,       # log prior
        nc.sbuf_tensor([1, 1], F32) as negmax,  # -max_k
        nc.sbuf_tensor([1, K], F32) as e,       # exp(log prior - max)
        nc.sbuf_tensor([1, 1], F32) as s,       # sum_k e
        nc.sbuf_tensor([1, 1], F32) as r,       # 1 / s
        nc.sbuf_tensor([1, K], F32) as p,       # softmax row
        nc.semaphore() as dma_sem,
        nc.semaphore() as v_sem,
        nc.semaphore() as a_sem,
        nc.Block() as block,
    ):

        @block.sync
        def _(sync):
            sync.dma_start(out=t[:], in_=lp[:]).then_inc(dma_sem, 16)
            # Wait fused onto the DMA instruction itself (saves one SP
            # dispatch vs a separate wait_ge).
            sync.dma_start(
                out=out[:],
                in_=p[:1, :].unsqueeze(1).broadcast_to([1, ROWS_PER_CORE, K]),
            )._wait_ge(v_sem, 3).then_inc(dma_sem, 16)

        @block.vector
        def _(vector):
            vector.wait_ge(dma_sem, 16)  # t loaded
            nc.vector.reduce_max(
                negmax[:], t[:], axis=mybir.AxisListType.X, negate=True
            ).then_inc(v_sem, 1)
            vector.wait_ge(a_sem, 1)  # e, s ready
            nc.vector.reciprocal(r[:], s[:]).then_inc(v_sem, 1)
            # Same-engine flush: r's writeback must retire before the next
            # instruction's scalar-ptr operand fetch (see hazard note above).
            vector.wait_ge(v_sem, 2)
            nc.vector.tensor_scalar_mul(p[:], e[:], r[:]).then_inc(v_sem, 1)

        @block.scalar
        def _(scalar):
            scalar.wait_ge(v_sem, 1)  # negmax ready
            # e = exp(t - max), s = sum_k e  (single fused ACT op)
            nc.scalar.activation(
                e[:],
                t[:],
                mybir.ActivationFunctionType.Exp,
                bias=negmax[:],
                scale=1.0,
                accum_out=s[:],
            ).then_inc(a_sem, 1)

    if lean and _references_const_table(nc):
        # Fail-safe: something pulled in a const AP after all — rebuild with
        # the const table properly initialized.
        return _build_nc(lean=False)
    return nc


def kernel(**inputs) -> np.ndarray:
    global LAST_RESULTS, _CACHED_NC
    log_prior = np.ascontiguousarray(
        np.asarray(inputs["log_codebook_prior"], dtype=np.float32)
    ).reshape(1, K)

    if _CACHED_NC is None:
        _CACHED_NC = _build_nc()
    nc = _CACHED_NC

    # B-dim data-parallel: every core holds the replicated prior and produces
    # its own 32-row shard of the [256, 64] output.
    in_maps = [{"log_prior": log_prior} for _ in range(N_CORES)]
    LAST_RESULTS = run_bass_kernel_spmd(nc, in_maps, list(range(N_CORES)))
    shards = [LAST_RESULTS.results[i]["out"] for i in range(N_CORES)]
    return np.ascontiguousarray(np.concatenate(shards, axis=0), dtype=np.float32)


if __name__ == "__main__":
    rng = np.random.default_rng(0)
    out = kernel(
        node_distributions=rng.standard_normal((20000, 32, 256), dtype=np.float32),
        batch_idx=rng.integers(0, B, size=(20000,)).astype(np.int32),
        codebook=rng.standard_normal((K, 256), dtype=np.float32),
        log_codebook_prior=np.zeros((K,), dtype=np.float32),
    )
    print(out.shape, out.dtype, out.min(), out.max())


# revision 15
# speedup vs baseline: 1.2492x; 1.0676x over previous
"""Bass/Trainium2 kernel for nn_BarycentricPooling_22660247453772.

Reference semantics
-------------------
The reference runs 30 log-domain sinkhorn iterations on each node's
[S=32, K=64] cost matrix, then one final (f, g) update pair, and builds the
transport-plan second marginal:

    hist[n, k] = sum_s exp((f[n,s] + g[n,k] - C[n,s,k]) / eps + log_a + log_b[k])

The final update computes  g[n,k] = -eps * lse_s(log_a + (f[n,s] - C[n,s,k]) / eps)
from the *same* f used in the histogram.  Substituting gives, exactly (in real
arithmetic, for every node n and any inputs):

    sum_s exp(log_pi[n,s,k])
      = exp(g[n,k]/eps + log_b[k]) * exp(lse_s(log_a + (f[n,s] - C[n,s,k])/eps))
      = exp(g[n,k]/eps + log_b[k]) * exp(-g[n,k]/eps)
      = exp(log_b[k])  =  softmax(log_codebook_prior)[k]

i.e. the final g half-iteration enforces the column-marginal constraint
exactly, so every per-node histogram equals the codebook prior b, the hist row
normalization divides by sum_k b_k = 1, every per-graph segment mean of
identical rows equals b, and the empty-graph fallback is b as well.  The whole
module output is therefore softmax(log_codebook_prior) broadcast to [B, K],
independent of node_distributions / batch_idx / codebook.  (Verified
numerically against the jax reference: max relative deviation 3.0e-5 on the
graded inputs — purely the reference's own fp32 round-off inside the exp/lse
telescoping.)

Kernel
------
So the roofline-optimal kernel computes softmax(log_codebook_prior) on-chip
and broadcasts it over the B=256 graph rows.  We shard the B dimension across
the 8 NeuronCores (32 graph rows per core, data-parallel SPMD): each core
  1. DMAs log_codebook_prior [1, 64] into SBUF,
  2. DVE reduce_max (negated) -> -max,
  3. ACT exp(t - max) with fused accumulate -> e, sum(e) in one instruction,
  4. DVE reciprocal -> 1/sum, then (after a same-engine semaphore flush)
     DVE tensor_scalar multiply -> softmax row [1, 64],
  5. DMAs the row with a free-dim-broadcast source AP to its [32, 64]
     output shard (the data-ready wait is fused onto the DMA instruction).
The host concatenates the 8 shards into the full [256, 64] output.

Raw Bass (manual semaphores) rather than TileContext: the walrus build in
this container rejects Tile's kernel-tail drain ("Too many sync wait
commands"), and this kernel's dependency chain is short enough to sync by
hand.
"""

from contextlib import ExitStack
from unittest import mock

import numpy as np

import concourse.bass as bass
from concourse import mybir
from concourse.bass_utils import run_bass_kernel_spmd

N_CORES = 8
B = 256  # number of graphs (hardcoded in the reference)
K = 64   # codebook size
ROWS_PER_CORE = B // N_CORES

F32 = mybir.dt.float32

# Kept for test-harness introspection.
LAST_RESULTS = None
_CACHED_NC = None


def _make_bass(lean: bool) -> bass.Bass:
    """Construct Bass; with lean=True, skip the init-time const-table memsets
    and the init all-engine barrier that only exists to order them.

    Bass.__init__ unconditionally memsets four const-AP scratch tensors on the
    Pool engine and then emits an all-engine barrier, so every engine's first
    real instruction waits ~750 ns for Pool.  This kernel never reads the
    const table (its only activation passes an AP bias, the one path that
    would pull in a const AP), and all of its cross-engine ordering is by
    explicit semaphores, so both are dead weight.  _build_nc verifies the
    no-const-reference assumption and rebuilds un-lean if it ever fails.
    The Block-exit barrier/drain (NEFF completion + sem lifecycle across
    repeat executions) is emitted outside the patch scope and is unaffected.
    """
    if not lean:
        return bass.Bass()
    with ExitStack() as st:
        st.enter_context(
            mock.patch.object(bass.BassGpSimd, "memset", lambda self, ap, c: None)
        )
        st.enter_context(
            mock.patch.object(
                bass.Bass, "all_engine_barrier", lambda self, *a, **k: None
            )
        )
        return bass.Bass()


def _references_const_table(nc: bass.Bass) -> bool:
    for bb in nc.m.functions[0].blocks:
        for ins in bb.instructions:
            if "const-" in str(ins):
                return True
    return False


def _build_nc(lean: bool = True) -> bass.Bass:
    nc = _make_bass(lean)
    lp = nc.declare_dram_parameter("log_prior", [1, K], F32, isOutput=False)
    out = nc.declare_dram_parameter("out", [ROWS_PER_CORE, K], F32, isOutput=True)

    # The DVE/ACT ops strictly alternate engines: a scalar-pointer operand
    # (activation bias/scale) read by the instruction right after its
    # same-engine producer fetches a stale value (engine-pipeline RAW hazard),
    # so every scalar-ptr producer here retires behind a cross-engine
    # semaphore wait before its consumer issues.
    with (
        nc.sbuf_tensor([1, K], F32) as t,       # log prior
        nc.sbuf_tensor([1, 1], F32) as negmax,  # -max_k
        nc.sbuf_tensor([1, K], F32) as e,       # exp(log prior - max)
        nc.sbuf_tensor([1, 1], F32) as s,       # sum_k e
        nc.sbuf_tensor([1, 1], F32) as r,       # 1 / s
        nc.sbuf_tensor([1, K], F32) as p,       # softmax row
        nc.semaphore() as dma_sem,
        nc.semaphore() as v_sem,
        nc.semaphore() as a_sem,
        nc.Block() as block,
    ):

        @block.sync
        def _(sync):
            sync.dma_start(out=t[:], in_=lp[:]).then_inc(dma_sem, 16)
            # Data-ready wait fused onto the DMA instruction itself (saves one
            # SP dispatch vs a separate wait_ge).  The completion then_inc is
            # structurally required (walrus crashes on a DMA with an empty
            # sync-update list; the final sem descriptor is also the HW's
            # write-completion guarantee).
            sync.dma_start(
                out=out[:],
                in_=p[:1, :].unsqueeze(1).broadcast_to([1, ROWS_PER_CORE, K]),
            )._wait_ge(v_sem, 3).then_inc(dma_sem, 16)

        # All waits are fused onto their consuming instruction (saves one
        # sequencer dispatch per wait; same semantics as a standalone
        # wait_ge, evaluated before dispatch and thus before any scalar-ptr
        # operand fetch).
        @block.vector
        def _(vector):
            nc.vector.reduce_max(
                negmax[:], t[:], axis=mybir.AxisListType.X, negate=True
            )._wait_ge(dma_sem, 16).then_inc(v_sem, 1)
            nc.vector.reciprocal(r[:], s[:])._wait_ge(a_sem, 1).then_inc(v_sem, 1)
            # The _wait_ge(v_sem, 2) is the same-engine flush: r's writeback
            # must retire before this instruction's scalar-ptr operand fetch
            # (see hazard note above).
            nc.vector.tensor_scalar_mul(p[:], e[:], r[:])._wait_ge(v_sem, 2).then_inc(
                v_sem, 1
            )

        @block.scalar
        def _(scalar):
            # e = exp(t - max), s = sum_k e  (single fused ACT op)
            nc.scalar.activation(
                e[:],
                t[:],
                mybir.ActivationFunctionType.Exp,
                bias=negmax[:],
                scale=1.0,
                accum_out=s[:],
            )._wait_ge(v_sem, 1).then_inc(a_sem, 1)

    if lean and _references_const_table(nc):
        # Fail-safe: something pulled in a const AP after all — rebuild with
        # the const table properly initialized.
        return _build_nc(lean=False)
    return nc


def kernel(**inputs) -> np.ndarray:
    global LAST_RESULTS, _CACHED_NC
    log_prior = np.ascontiguousarray(
        np.asarray(inputs["log_codebook_prior"], dtype=np.float32)
    ).reshape(1, K)

    if _CACHED_NC is None:
        _CACHED_NC = _build_nc()
    nc = _CACHED_NC

    # B-dim data-parallel: every core holds the replicated prior and produces
    # its own 32-row shard of the [256, 64] output.
    in_maps = [{"log_prior": log_prior} for _ in range(N_CORES)]
    LAST_RESULTS = run_bass_kernel_spmd(nc, in_maps, list(range(N_CORES)))
    shards = [LAST_RESULTS.results[i]["out"] for i in range(N_CORES)]
    return np.ascontiguousarray(np.concatenate(shards, axis=0), dtype=np.float32)


if __name__ == "__main__":
    rng = np.random.default_rng(0)
    out = kernel(
        node_distributions=rng.standard_normal((20000, 32, 256), dtype=np.float32),
        batch_idx=rng.integers(0, B, size=(20000,)).astype(np.int32),
        codebook=rng.standard_normal((K, 256), dtype=np.float32),
        log_codebook_prior=np.zeros((K,), dtype=np.float32),
    )
    print(out.shape, out.dtype, out.min(), out.max())
